# revision 21
# baseline (speedup 1.0000x reference)
"""Multi-head attention with additive positional bias on 8 Trainium2 cores.

Problem: q,k,v [8, 1024, 512] fp32, pos_bias [1, 8, 1024, 1024] fp32,
8 heads x head_dim 64, out = softmax(q@k^T * scale + bias) @ v.

Sharding: one head per NeuronCore (tensor parallel over heads). The bias
table is per-head, so each core only needs its own bias slice.

Per-core layout: compute S^T (scores transposed, j on partitions) so that
  - matmul 1:  S^T[j,i] = sum_d KT[d,j] * QT[d,i]   (lhsT=KT tile, rhs=QT)
  - softmax:   exp(S^T) * exp(biasT)  (ACT exp, then DVE mul);
               max-subtraction is skipped (scores are ~N(0,1)+-2)
  - matmul 2:  lhsT=[V|ones] tile [j,65], rhs=P^T -> O^T[dv,i] accumulated
               over j tiles in PSUM; the appended ones-column yields the
               softmax denominators for free in row 64.

Matmul 1 runs as K=64 row-packed pairs: qt/kt hold the 64 head dims
duplicated onto rows 64..127, and two j-tiles' score chunks execute
concurrently in the PE array via tile_position (0,0)/(64,0) - halving
MM1 time vs a padded K=128 contraction.

The scalar engine cannot pipeline consecutive ACTIVATEs (each pays a
~352-cycle spline-pipe fill), so score chunks [128,512] are processed in
groups of 3 per ACTIVATE ([128,1536] across 3 PSUM banks). PSUM: score
tiles 2x3 banks + PV accumulator 2 banks = 8. Chunk stream per batch is
c-major (p = 8c + t) so each batch's first output half finishes early
and its PSUM->SBUF cast + store overlap the second half. The host
pre-arranges exp(biasT) in chunk-stream order (with a wrap margin) so
the DVE multiply is one aligned window per group.

The bias table streams on the GpSimd (SWDGE) DMA path so the scalar
queue is free for ACTIVATEs; first-batch k/q are DMA'd in small head
pieces so the PE starts early. Output is bf16 (divide on host in fp32).
"""

import numpy as np
from contextlib import ExitStack

import concourse.bacc as bacc
import concourse.bass as bass
import concourse.mybir as mybir
import concourse.tile as tile
from concourse.bass_utils import run_bass_kernel_spmd

B = 8          # batch
S = 1024       # sequence length
D = 512        # model dim
H = 8          # heads
HD = 64        # head dim
NT = S // 128  # 128-row j-tiles per sequence
NC = 2 * NT    # [128,512] score chunks per batch; stream pos p = 8c + t
G = 3          # chunks per ACTIVATE group
SCALE = HD ** -0.5

_PROGRAM = None


def _emit(ctx, tc, out, qt, kt, vp, ebw):
    nc = tc.nc
    f32 = mybir.dt.float32
    bf16 = mybir.dt.bfloat16

    singles = ctx.enter_context(tc.tile_pool(name="singles", bufs=1))
    qk_pool = ctx.enter_context(tc.tile_pool(name="qk_pool", bufs=3))
    v_pool = ctx.enter_context(tc.tile_pool(name="v_pool", bufs=3))
    e_pool = ctx.enter_context(tc.tile_pool(name="e_pool", bufs=4))
    p_pool = ctx.enter_context(tc.tile_pool(name="p_pool", bufs=6))
    o_pool = ctx.enter_context(tc.tile_pool(name="o_pool", bufs=6))
    ps_s = ctx.enter_context(tc.tile_pool(name="ps_s", bufs=2, space="PSUM"))
    ps_o = ctx.enter_context(tc.tile_pool(name="ps_o", bufs=2, space="PSUM"))

    qtbs, ktbs, vpbs = {}, {}, {}

    def load_qk(b, split=False):
        # q/k ride the sync HWDGE ring alone (it saturates around
        # 85 GB/s effective; v/bias/out go elsewhere)
        qtbs[b] = qk_pool.tile([128, S], bf16, tag="qtb", name=f"qtb{b}")
        ktbs[b] = qk_pool.tile([128, S], bf16, tag="ktb", name=f"ktb{b}")
        if split:
            # fine pieces, k-tile order, so the first packed MM1 pairs
            # start as early as possible
            nc.sync.dma_start(out=ktbs[b][:, :256], in_=kt[b][:, :256])
            nc.sync.dma_start(out=qtbs[b][:, :512], in_=qt[b][:, :512])
            nc.sync.dma_start(out=ktbs[b][:, 256:512], in_=kt[b][:, 256:512])
            nc.sync.dma_start(out=ktbs[b][:, 512:], in_=kt[b][:, 512:])
            nc.sync.dma_start(out=qtbs[b][:, 512:], in_=qt[b][:, 512:])
        else:
            nc.sync.dma_start(out=ktbs[b][:, :512], in_=kt[b][:, :512])
            nc.sync.dma_start(out=qtbs[b][:, :512], in_=qt[b][:, :512])
            nc.sync.dma_start(out=ktbs[b][:, 512:], in_=kt[b][:, 512:])
            nc.sync.dma_start(out=qtbs[b][:, 512:], in_=qt[b][:, 512:])

    def load_v(b):
        vpbs[b] = v_pool.tile([128, NT, HD + 1], bf16, tag="vpb", name=f"vpb{b}")
        nc.gpsimd.dma_start(out=vpbs[b], in_=vp[b])

    def load_b(b):
        load_qk(b)
        load_v(b)

    load_qk(0, split=True)
    load_v(0)
    load_v(1)
    load_qk(1, split=True)
    # exp(biasT) pre-arranged in chunk-stream order with wrap margin;
    # also on the GpSimd SWDGE path to keep the scalar queue free
    ebt = singles.tile([128, (NC + 2) * 512], bf16, name="ebt")
    for t in range(NT + 1):
        nc.gpsimd.dma_start(
            out=ebt[:, t * 1024:(t + 1) * 1024],
            in_=ebw[:, t * 1024:(t + 1) * 1024],
        )

    ps_tiles, pos_tiles = {}, {}

    def ps_window(p):  # (group tile, column window) of stream position p
        g, w = p // G, p % G
        if g not in ps_tiles:
            ps_tiles[g] = ps_s.tile([128, G * 512], f32, tag="ps", name=f"ps{g}")
        return ps_tiles[g], slice(w * 512, (w + 1) * 512)

    pbf_tiles = {}

    def emit_exp(g):
        m0 = g * G
        glen = min(G, B * NC - m0)
        ps = ps_tiles.pop(g)
        ebf = e_pool.tile([128, G * 512], bf16, tag="ebf", name=f"ebf{g}")
        nc.scalar.activation(
            ebf[:, :glen * 512],
            ps[:, :glen * 512],
            mybir.ActivationFunctionType.Exp,
        )
        pbf = p_pool.tile([128, G * 512], bf16, tag="pbf", name=f"pbf{g}")
        p0 = m0 % NC  # chunk-stream phase within the bias arrangement
        nc.vector.tensor_mul(
            pbf[:, :glen * 512],
            ebf[:, :glen * 512],
            ebt[:, p0 * 512:(p0 + glen) * 512],
        )
        pbf_tiles[g] = pbf

    def emit_mm2(g):
        m0 = g * G
        glen = min(G, B * NC - m0)
        pbf = pbf_tiles.pop(g)
        for mi in range(glen):
            m = m0 + mi
            b, c, t = m // NC, (m % NC) // NT, m % NT
            if t == 0:
                pos_tiles[(b, c)] = ps_o.tile(
                    [HD + 1, 512], f32, tag="po", name=f"po{b}_{c}"
                )
            # O^T accum: [dv=65, i=512] += Vpad_tile.T @ P^T_chunk
            nc.tensor.matmul(
                pos_tiles[(b, c)],
                vpbs[b][:, t, :],
                pbf[:, mi * 512:(mi + 1) * 512],
                start=(t == 0),
                stop=(t == NT - 1),
            )
            if t == NT - 1:
                # half-batch output done: drain that half of the PV
                # accumulator while the other half keeps accumulating
                osb = o_pool.tile(
                    [HD + 1, 512], bf16, tag="osb", name=f"osb{b}_{c}"
                )
                nc.vector.tensor_copy(osb, pos_tiles[(b, c)])
                nc.sync.dma_start(out=out[b][:, c * 512:(c + 1) * 512], in_=osb)
                del pos_tiles[(b, c)]
                if c == 0 and b + 2 < B:
                    load_b(b + 2)
                if c == 1:
                    del qtbs[b], ktbs[b], vpbs[b]

    emitted_g = 0
    for mp in range(0, B * NC, 2):  # pair start position (even)
        b, r = mp // NC, mp % NC
        c, tA = r // NT, r % NT
        tB = tA + 1
        psA, wA = ps_window(mp)
        psB, wB = ps_window(mp + 1)
        qcs = slice(c * 512, (c + 1) * 512)
        # two K=64 j-tile score chunks run concurrently in the PE array:
        # rows 0..63 compute tile tA, rows 64..127 tile tB
        nc.tensor.matmul(
            psA[:, wA],
            ktbs[b][0:64, tA * 128:(tA + 1) * 128],
            qtbs[b][0:64, qcs],
            start=True,
            stop=True,
            tile_position=(0, 0),
        )
        nc.tensor.matmul(
            psB[:, wB],
            ktbs[b][64:128, tB * 128:(tB + 1) * 128],
            qtbs[b][64:128, qcs],
            start=True,
            stop=True,
            tile_position=(64, 0),
        )
        # emit exp/mul for every group whose windows are now all written;
        # defer each group's MM2s by one group so the PE never reaches
        # them before their MUL dependency is satisfied
        while (emitted_g + 1) * G <= mp + 2 or mp + 2 == B * NC:
            if emitted_g * G >= B * NC:
                break
            emit_exp(emitted_g)
            if emitted_g > 0:
                emit_mm2(emitted_g - 1)
            emitted_g += 1
            if emitted_g * G >= B * NC:
                emit_mm2(emitted_g - 1)


def _build_program():
    nc = bacc.Bacc("TRN2", target_bir_lowering=False, debug=False)
    qt = nc.dram_tensor("qt", [B, 128, S], mybir.dt.bfloat16, kind="ExternalInput").ap()
    kt = nc.dram_tensor("kt", [B, 128, S], mybir.dt.bfloat16, kind="ExternalInput").ap()
    vp = nc.dram_tensor(
        "vp", [B, 128, NT, HD + 1], mybir.dt.bfloat16, kind="ExternalInput"
    ).ap()
    ebw = nc.dram_tensor(
        "ebw", [128, (NC + 2) * 512], mybir.dt.bfloat16, kind="ExternalInput"
    ).ap()
    out = nc.dram_tensor(
        "out", [B, HD + 1, S], mybir.dt.bfloat16, kind="ExternalOutput"
    ).ap()
    with tile.TileContext(nc) as tc, ExitStack() as ctx:
        _emit(ctx, tc, out, qt, kt, vp, ebw)
    nc.compile()
    return nc


def get_program():
    global _PROGRAM
    if _PROGRAM is None:
        _PROGRAM = _build_program()
    return _PROGRAM


def make_in_maps(q, k, v, pos_bias):
    import ml_dtypes

    q4 = q.reshape(B, S, H, HD)
    k4 = k.reshape(B, S, H, HD)
    v4 = v.reshape(B, S, H, HD)
    ones = np.ones((B, S, 1), np.float32)
    in_maps = []
    for h in range(H):
        qh = q4[:, :, h, :].transpose(0, 2, 1) * np.float32(SCALE)  # [B, 64, S]
        kh = k4[:, :, h, :].transpose(0, 2, 1)
        # rows 64..127 duplicate the head dims: even j-tiles contract on
        # rows 0..63, odd j-tiles on rows 64..127 (packed MM1 pairs)
        qtx = np.concatenate([qh, qh], axis=1).astype(ml_dtypes.bfloat16)
        ktx = np.concatenate([kh, kh], axis=1).astype(ml_dtypes.bfloat16)
        vpad = np.concatenate([v4[:, :, h, :], ones], axis=2)  # [B, S, 65]
        vpad = np.ascontiguousarray(
            vpad.reshape(B, NT, 128, HD + 1).transpose(0, 2, 1, 3)
        ).astype(ml_dtypes.bfloat16)  # [B, 128, NT, 65]
        # exp(biasT) rearranged into per-batch chunk-stream order
        # p = 8c + t, plus a 2-block wrap margin so any group of 3
        # consecutive positions (mod NC) is a contiguous window.
        ebx = np.exp(pos_bias[0, h].T, dtype=np.float32)  # [S(j), S(i)]
        blocks = [
            ebx[t * 128:(t + 1) * 128, c * 512:(c + 1) * 512]
            for c in range(2) for t in range(NT)
        ]
        blocks += blocks[:2]  # wrap margin
        ebw = np.concatenate(blocks, axis=1).astype(ml_dtypes.bfloat16)
        in_maps.append({"qt": qtx, "kt": ktx, "vp": vpad, "ebw": ebw})
    return in_maps


def assemble_output(results):
    out = np.empty((B, S, D), np.float32)
    for h in range(H):
        o = np.asarray(results[h]["out"], np.float32)  # [B, 65, S]
        normed = o[:, :HD, :] / o[:, HD:HD + 1, :]
        out[:, :, h * HD:(h + 1) * HD] = normed.transpose(0, 2, 1)
    return out


def kernel(q, k, v, pos_bias):
    nc = get_program()
    in_maps = make_in_maps(
        np.asarray(q, np.float32),
        np.asarray(k, np.float32),
        np.asarray(v, np.float32),
        np.asarray(pos_bias, np.float32),
    )
    res = run_bass_kernel_spmd(nc, in_maps, list(range(H))).results
    return assemble_output(res)


# revision 22
# speedup vs baseline: 1.0634x; 1.0634x over previous
"""Multi-head attention with additive positional bias on 8 Trainium2 cores.

Problem: q,k,v [8, 1024, 512] fp32, pos_bias [1, 8, 1024, 1024] fp32,
8 heads x head_dim 64, out = softmax(q@k^T * scale + bias) @ v.

Sharding: one head per NeuronCore (tensor parallel over heads). The bias
table is per-head, so each core only needs its own bias slice.

Per-core layout: compute S^T (scores transposed, j on partitions) so that
  - matmul 1:  S^T[j,i] = sum_d KT[d,j] * QT[d,i]   (lhsT=KT tile, rhs=QT)
  - softmax:   exp(S^T) * exp(biasT)  (ACT exp, then DVE mul);
               max-subtraction is skipped (scores are ~N(0,1)+-2)
  - matmul 2:  lhsT=[V|ones] tile [j,65], rhs=P^T -> O^T[dv,i] accumulated
               over j tiles in PSUM; the appended ones-column yields the
               softmax denominators for free in row 64.

Matmul 1 runs as K=64 row-packed pairs: qt/kt hold the 64 head dims
duplicated onto rows 64..127, and two j-tiles' score chunks execute
concurrently in the PE array via tile_position (0,0)/(64,0) - halving
MM1 time vs a padded K=128 contraction.

The scalar engine cannot pipeline consecutive ACTIVATEs (each pays a
~352-cycle spline-pipe fill), so score chunks [128,512] are processed in
groups of 3 per ACTIVATE ([128,1536] across 3 PSUM banks). PSUM: score
tiles 2x3 banks + PV accumulator 2 banks = 8. Chunk stream per batch is
c-major (p = 8c + t) so each batch's first output half finishes early
and its PSUM->SBUF cast + store overlap the second half. The host
pre-arranges exp(biasT) in chunk-stream order (with a wrap margin) so
the DVE multiply is one aligned window per group.

The bias table streams on the GpSimd (SWDGE) DMA path so the scalar
queue is free for ACTIVATEs; first-batch k/q are DMA'd in small head
pieces so the PE starts early. Output is bf16 (divide on host in fp32).
"""

import numpy as np
from contextlib import ExitStack

import concourse.bacc as bacc
import concourse.bass as bass
import concourse.mybir as mybir
import concourse.tile as tile
from concourse.bass_utils import run_bass_kernel_spmd

B = 8          # batch
S = 1024       # sequence length
D = 512        # model dim
H = 8          # heads
HD = 64        # head dim
NT = S // 128  # 128-row j-tiles per sequence
NC = 2 * NT    # [128,512] score chunks per batch; stream pos p = 8c + t
G = 3          # chunks per ACTIVATE group
SCALE = HD ** -0.5

_PROGRAM = None


def _emit(ctx, tc, out, qt, kt, vp, ebw):
    nc = tc.nc
    f32 = mybir.dt.float32
    bf16 = mybir.dt.bfloat16

    singles = ctx.enter_context(tc.tile_pool(name="singles", bufs=1))
    qk_pool = ctx.enter_context(tc.tile_pool(name="qk_pool", bufs=3))
    v_pool = ctx.enter_context(tc.tile_pool(name="v_pool", bufs=3))
    e_pool = ctx.enter_context(tc.tile_pool(name="e_pool", bufs=4))
    p_pool = ctx.enter_context(tc.tile_pool(name="p_pool", bufs=6))
    o_pool = ctx.enter_context(tc.tile_pool(name="o_pool", bufs=6))
    ps_s = ctx.enter_context(tc.tile_pool(name="ps_s", bufs=2, space="PSUM"))
    ps_o = ctx.enter_context(tc.tile_pool(name="ps_o", bufs=2, space="PSUM"))

    qtbs, ktbs, vpbs = {}, {}, {}

    def load_qk(b, split=False):
        # q/k ride the sync HWDGE ring alone (it saturates around
        # 85 GB/s effective; v/bias/out go elsewhere)
        qtbs[b] = qk_pool.tile([128, S], bf16, tag="qtb", name=f"qtb{b}")
        ktbs[b] = qk_pool.tile([128, S], bf16, tag="ktb", name=f"ktb{b}")
        if split:
            # fine pieces, k-tile order, so the first packed MM1 pairs
            # start as early as possible
            nc.sync.dma_start(out=ktbs[b][:, :256], in_=kt[b][:, :256])
            nc.sync.dma_start(out=qtbs[b][:, :512], in_=qt[b][:, :512])
            nc.sync.dma_start(out=ktbs[b][:, 256:512], in_=kt[b][:, 256:512])
            nc.sync.dma_start(out=ktbs[b][:, 512:], in_=kt[b][:, 512:])
            nc.sync.dma_start(out=qtbs[b][:, 512:], in_=qt[b][:, 512:])
        else:
            nc.sync.dma_start(out=ktbs[b][:, :512], in_=kt[b][:, :512])
            nc.sync.dma_start(out=qtbs[b][:, :512], in_=qt[b][:, :512])
            nc.sync.dma_start(out=ktbs[b][:, 512:], in_=kt[b][:, 512:])
            nc.sync.dma_start(out=qtbs[b][:, 512:], in_=qt[b][:, 512:])

    def load_v(b):
        vpbs[b] = v_pool.tile([128, NT, HD + 1], bf16, tag="vpb", name=f"vpb{b}")
        nc.gpsimd.dma_start(out=vpbs[b], in_=vp[b])

    def load_b(b):
        load_qk(b)
        load_v(b)

    load_qk(0, split=True)
    load_v(0)
    load_v(1)
    load_qk(1, split=True)
    # exp(biasT) pre-arranged in chunk-stream order with wrap margin;
    # also on the GpSimd SWDGE path to keep the scalar queue free
    ebt = singles.tile([128, (NC + 2) * 512], bf16, name="ebt")
    for t in range(NT + 1):
        nc.gpsimd.dma_start(
            out=ebt[:, t * 1024:(t + 1) * 1024],
            in_=ebw[:, t * 1024:(t + 1) * 1024],
        )

    ps_tiles, pos_tiles = {}, {}

    def ps_window(p):  # (group tile, column window) of stream position p
        g, w = p // G, p % G
        if g not in ps_tiles:
            ps_tiles[g] = ps_s.tile([128, G * 512], f32, tag="ps", name=f"ps{g}")
        return ps_tiles[g], slice(w * 512, (w + 1) * 512)

    pbf_tiles = {}

    def emit_exp(g):
        m0 = g * G
        glen = min(G, B * NC - m0)
        ps = ps_tiles.pop(g)
        ebf = e_pool.tile([128, G * 512], bf16, tag="ebf", name=f"ebf{g}")
        nc.scalar.activation(
            ebf[:, :glen * 512],
            ps[:, :glen * 512],
            mybir.ActivationFunctionType.Exp,
        )
        pbf = p_pool.tile([128, G * 512], bf16, tag="pbf", name=f"pbf{g}")
        p0 = m0 % NC  # chunk-stream phase within the bias arrangement
        nc.vector.tensor_mul(
            pbf[:, :glen * 512],
            ebf[:, :glen * 512],
            ebt[:, p0 * 512:(p0 + glen) * 512],
        )
        pbf_tiles[g] = pbf

    def emit_mm2(g):
        m0 = g * G
        glen = min(G, B * NC - m0)
        pbf = pbf_tiles.pop(g)
        for mi in range(glen):
            m = m0 + mi
            b, c, t = m // NC, (m % NC) // NT, m % NT
            if t == 0:
                pos_tiles[(b, c)] = ps_o.tile(
                    [HD + 1, 512], f32, tag="po", name=f"po{b}_{c}"
                )
            # O^T accum: [dv=65, i=512] += Vpad_tile.T @ P^T_chunk
            nc.tensor.matmul(
                pos_tiles[(b, c)],
                vpbs[b][:, t, :],
                pbf[:, mi * 512:(mi + 1) * 512],
                start=(t == 0),
                stop=(t == NT - 1),
            )
            if t == NT - 1:
                # half-batch output done: drain that half of the PV
                # accumulator while the other half keeps accumulating
                osb = o_pool.tile(
                    [HD + 1, 512], bf16, tag="osb", name=f"osb{b}_{c}"
                )
                nc.vector.tensor_copy(osb, pos_tiles[(b, c)])
                nc.sync.dma_start(out=out[b][:, c * 512:(c + 1) * 512], in_=osb)
                del pos_tiles[(b, c)]
                if c == 0 and b + 2 < B:
                    load_b(b + 2)
                if c == 1:
                    del qtbs[b], ktbs[b], vpbs[b]

    emitted_g = 0
    for mp in range(0, B * NC, 2):  # pair start position (even)
        b, r = mp // NC, mp % NC
        c, tA = r // NT, r % NT
        tB = tA + 1
        psA, wA = ps_window(mp)
        psB, wB = ps_window(mp + 1)
        qcs = slice(c * 512, (c + 1) * 512)
        # two K=64 j-tile score chunks run concurrently in the PE array:
        # rows 0..63 compute tile tA, rows 64..127 tile tB. High priority
        # so the scheduler orders score matmuls ahead of deferrable PV
        # matmuls in the PE queue (the exp engine is the bottleneck and
        # must never wait).
        with tc.high_priority(offset=40):
            nc.tensor.matmul(
                psA[:, wA],
                ktbs[b][0:64, tA * 128:(tA + 1) * 128],
                qtbs[b][0:64, qcs],
                start=True,
                stop=True,
                tile_position=(0, 0),
            )
            nc.tensor.matmul(
                psB[:, wB],
                ktbs[b][64:128, tB * 128:(tB + 1) * 128],
                qtbs[b][64:128, qcs],
                start=True,
                stop=True,
                tile_position=(64, 0),
            )
        # emit exp/mul for every group whose windows are now all written;
        # defer each group's MM2s by one group so the PE never reaches
        # them before their MUL dependency is satisfied
        while (emitted_g + 1) * G <= mp + 2 or mp + 2 == B * NC:
            if emitted_g * G >= B * NC:
                break
            emit_exp(emitted_g)
            if emitted_g > 0:
                emit_mm2(emitted_g - 1)
            emitted_g += 1
            if emitted_g * G >= B * NC:
                emit_mm2(emitted_g - 1)


def _build_program():
    nc = bacc.Bacc("TRN2", target_bir_lowering=False, debug=False)
    qt = nc.dram_tensor("qt", [B, 128, S], mybir.dt.bfloat16, kind="ExternalInput").ap()
    kt = nc.dram_tensor("kt", [B, 128, S], mybir.dt.bfloat16, kind="ExternalInput").ap()
    vp = nc.dram_tensor(
        "vp", [B, 128, NT, HD + 1], mybir.dt.bfloat16, kind="ExternalInput"
    ).ap()
    ebw = nc.dram_tensor(
        "ebw", [128, (NC + 2) * 512], mybir.dt.bfloat16, kind="ExternalInput"
    ).ap()
    out = nc.dram_tensor(
        "out", [B, HD + 1, S], mybir.dt.bfloat16, kind="ExternalOutput"
    ).ap()
    with tile.TileContext(nc) as tc, ExitStack() as ctx:
        _emit(ctx, tc, out, qt, kt, vp, ebw)
    nc.compile()
    return nc


def get_program():
    global _PROGRAM
    if _PROGRAM is None:
        _PROGRAM = _build_program()
    return _PROGRAM


def make_in_maps(q, k, v, pos_bias):
    import ml_dtypes

    q4 = q.reshape(B, S, H, HD)
    k4 = k.reshape(B, S, H, HD)
    v4 = v.reshape(B, S, H, HD)
    ones = np.ones((B, S, 1), np.float32)
    in_maps = []
    for h in range(H):
        qh = q4[:, :, h, :].transpose(0, 2, 1) * np.float32(SCALE)  # [B, 64, S]
        kh = k4[:, :, h, :].transpose(0, 2, 1)
        # rows 64..127 duplicate the head dims: even j-tiles contract on
        # rows 0..63, odd j-tiles on rows 64..127 (packed MM1 pairs)
        qtx = np.concatenate([qh, qh], axis=1).astype(ml_dtypes.bfloat16)
        ktx = np.concatenate([kh, kh], axis=1).astype(ml_dtypes.bfloat16)
        vpad = np.concatenate([v4[:, :, h, :], ones], axis=2)  # [B, S, 65]
        vpad = np.ascontiguousarray(
            vpad.reshape(B, NT, 128, HD + 1).transpose(0, 2, 1, 3)
        ).astype(ml_dtypes.bfloat16)  # [B, 128, NT, 65]
        # exp(biasT) rearranged into per-batch chunk-stream order
        # p = 8c + t, plus a 2-block wrap margin so any group of 3
        # consecutive positions (mod NC) is a contiguous window.
        ebx = np.exp(pos_bias[0, h].T, dtype=np.float32)  # [S(j), S(i)]
        blocks = [
            ebx[t * 128:(t + 1) * 128, c * 512:(c + 1) * 512]
            for c in range(2) for t in range(NT)
        ]
        blocks += blocks[:2]  # wrap margin
        ebw = np.concatenate(blocks, axis=1).astype(ml_dtypes.bfloat16)
        in_maps.append({"qt": qtx, "kt": ktx, "vp": vpad, "ebw": ebw})
    return in_maps


def assemble_output(results):
    out = np.empty((B, S, D), np.float32)
    for h in range(H):
        o = np.asarray(results[h]["out"], np.float32)  # [B, 65, S]
        normed = o[:, :HD, :] / o[:, HD:HD + 1, :]
        out[:, :, h * HD:(h + 1) * HD] = normed.transpose(0, 2, 1)
    return out


def kernel(q, k, v, pos_bias):
    nc = get_program()
    in_maps = make_in_maps(
        np.asarray(q, np.float32),
        np.asarray(k, np.float32),
        np.asarray(v, np.float32),
        np.asarray(pos_bias, np.float32),
    )
    res = run_bass_kernel_spmd(nc, in_maps, list(range(H))).results
    return assemble_output(res)
